# revision 13
# baseline (speedup 1.0000x reference)
"""Trainium2 Bass kernel for nn_AttentionBlock (GroupNorm -> MHA(8 heads, s=4096) -> proj -> residual).

Sharding: 8 cores = 2 batches x 4 query-token slices (1024 tokens each).
Each core computes GroupNorm + full K/V for its batch (redundant across the
4 cores of a batch), Q only for its token slice, streaming softmax attention
for all 8 heads over its slice, projection + residual for its slice.
Output slices are disjoint -> no cross-core reduction.

Self-contained: hardcodes shapes (x: (2,256,64,64) f32) and reads nothing
from /root/problem.
"""

import os
import sys
import math
import numpy as np

sys.path.insert(0, "/opt/trn_rl_repo")

import ml_dtypes  # noqa: E402

BF16 = ml_dtypes.bfloat16

# ---- problem constants (hardcoded) ----
B, C, H, W = 2, 256, 64, 64
S = H * W            # 4096 tokens
NH, HD = 8, 32       # heads, head dim
GROUPS = 32          # groupnorm groups
CPG = C // GROUPS    # 8 channels / group
EPS = 1e-5
NCORES = 8
NSL = 4              # token slices per batch
SL = S // NSL        # 1024 tokens per core
NT = C // 128        # 2 channel tiles
TCH = S // 128       # 32 key/t chunks
NP = SL // 512       # 2 moving pieces per psum row-block

_PROGRAM = None      # (nc, meta) cache
LAST_RESULTS = None  # BassKernelResults of last run (for test introspection)

# softmax exp split between ACT (hw spline) and a custom DVE cubic-poly op.
# scores are stored pre-scaled by 1/4; ACT applies scale=4, the DVE poly
# evaluates exp(4s) ~= (1 + s(1 + s(c1 + s*c0)))^4 (fit on |score|<=0.85).
EXP4_C0 = 0.16665777
EXP4_C1 = 0.5015184
DVE_EXP_PAT = (0, 2, 4, 6)   # chunk c goes to DVE iff c % 9 in this set
_EXP4_OP = None


def _register_exp4():
    """Register the ANT_EXP4 custom DVE op (runtime append to dve_ops.OPS)."""
    global _EXP4_OP
    if _EXP4_OP is not None:
        return _EXP4_OP
    from concourse import dve_ops
    from concourse.dve_spec import Spec, Src0, C0, C1, One, lower
    from concourse.dve_uop import DveOpSpec
    from concourse.dve_ops import DveOp

    for op in dve_ops.OPS:
        if op.name == "ANT_EXP4":
            _EXP4_OP = op
            return op

    def _ref(in0, in1, s0, s1, imm2):
        t1 = s1 + in0 * s0
        t2 = 1.0 + in0 * t1
        t3 = 1.0 + in0 * t2
        t4 = t3 * t3
        return (t4 * t4).astype(np.float32)

    t1 = C1 + Src0 * C0
    t2 = One + Src0 * t1
    t3 = One + Src0 * t2
    t4 = t3 * t3
    spec = Spec(body=t4 * t4, reference=_ref)
    shas = {
        ver: DveOpSpec(
            name="ANT_EXP4", opcode=0, uops=lower(spec, ver=ver), rd1_en=False
        ).sha(ver)
        for ver in ("v3", "v4")
    }
    op = DveOp("ANT_EXP4", spec, subdim=False, uops_sha=shas)
    dve_ops.OPS.append(op)
    dve_ops._SUB_OPCODE_FOR_NAME[op.name] = (
        dve_ops._CUSTOM_DVE_ROW_BASE + len(dve_ops.OPS) - 1
    )
    dve_ops.CUSTOM_DVE_SPECS[op.name] = spec
    _EXP4_OP = op
    return op


def _build_program():
    import concourse.bass as bass
    import concourse.tile as tile
    from concourse import bacc, mybir

    f32 = mybir.dt.float32
    bf16 = mybir.dt.bfloat16
    Alu = mybir.AluOpType
    Act = mybir.ActivationFunctionType

    nc = bacc.Bacc(
        "TRN2",
        target_bir_lowering=False,
        debug=False,
        enable_asserts=False,
        num_devices=NCORES,
    )

    # ---- DRAM I/O ----
    x_full = nc.dram_tensor("x_full", [C, S], f32, kind="ExternalInput").ap()
    x_sl = nc.dram_tensor("x_sl", [C, SL], f32, kind="ExternalInput").ap()
    wq_t = nc.dram_tensor("wq_t", [C, C], bf16, kind="ExternalInput").ap()
    wk_t = nc.dram_tensor("wk_t", [C, C], bf16, kind="ExternalInput").ap()
    wv_t = nc.dram_tensor("wv_t", [C, C], bf16, kind="ExternalInput").ap()
    p_t = nc.dram_tensor("p_t", [C, C], bf16, kind="ExternalInput").ap()
    bq_d = nc.dram_tensor("bq", [C, 1], f32, kind="ExternalInput").ap()
    bk_d = nc.dram_tensor("bk", [C, 1], f32, kind="ExternalInput").ap()
    bvr_d = nc.dram_tensor("bv_row", [1, C], f32, kind="ExternalInput").ap()
    pb_d = nc.dram_tensor("pb", [C, 1], f32, kind="ExternalInput").ap()
    gnw_d = nc.dram_tensor("gnw", [C, 1], f32, kind="ExternalInput").ap()
    gnb_d = nc.dram_tensor("gnb", [C, 1], f32, kind="ExternalInput").ap()
    g8_d = nc.dram_tensor("g8", [128, 16], f32, kind="ExternalInput").ap()
    g8t_d = nc.dram_tensor("g8t", [16, 128], f32, kind="ExternalInput").ap()
    out_d = nc.dram_tensor("out_sl", [C, SL], f32, kind="ExternalOutput").ap()

    with tile.TileContext(nc) as tc:
        with tc.tile_pool(name="consts", bufs=1) as consts, \
             tc.tile_pool(name="data", bufs=1) as data, \
             tc.tile_pool(name="work", bufs=3) as work:

            # ---------- load inputs ----------
            x_sb = data.tile([128, NT, S], f32)
            xsl_sb = data.tile([128, NT, SL], f32)
            for t in range(NT):
                nc.sync.dma_start(out=x_sb[:, t, :], in_=x_full[t * 128:(t + 1) * 128, :])
                nc.sync.dma_start(out=xsl_sb[:, t, :], in_=x_sl[t * 128:(t + 1) * 128, :])

            wq_sb = consts.tile([128, NT, C], bf16)
            wk_sb = consts.tile([128, NT, C], bf16)
            wv_sb = consts.tile([128, NT, C], bf16)
            p_sb = consts.tile([128, NT, C], bf16)
            bq_sb = consts.tile([128, NT, 1], f32)
            bk_sb = consts.tile([128, NT, 1], f32)
            pb_sb = consts.tile([128, NT, 1], f32)
            gnw_sb = consts.tile([128, NT, 1], f32)
            gnb_sb = consts.tile([128, NT, 1], f32)
            for t in range(NT):
                sl_ = slice(t * 128, (t + 1) * 128)
                nc.sync.dma_start(out=wq_sb[:, t, :], in_=wq_t[sl_, :])
                nc.sync.dma_start(out=wk_sb[:, t, :], in_=wk_t[sl_, :])
                nc.sync.dma_start(out=wv_sb[:, t, :], in_=wv_t[sl_, :])
                nc.sync.dma_start(out=p_sb[:, t, :], in_=p_t[sl_, :])
                nc.sync.dma_start(out=bq_sb[:, t, :], in_=bq_d[sl_, :])
                nc.sync.dma_start(out=bk_sb[:, t, :], in_=bk_d[sl_, :])
                nc.sync.dma_start(out=pb_sb[:, t, :], in_=pb_d[sl_, :])
                nc.sync.dma_start(out=gnw_sb[:, t, :], in_=gnw_d[sl_, :])
                nc.sync.dma_start(out=gnb_sb[:, t, :], in_=gnb_d[sl_, :])
            bvr_sb = consts.tile([1, C], f32)
            nc.sync.dma_start(out=bvr_sb[:], in_=bvr_d[:, :])
            g8_sb = consts.tile([128, 16], f32)
            nc.sync.dma_start(out=g8_sb[:], in_=g8_d[:, :])
            g8t_sb = consts.tile([16, 128], f32)
            nc.sync.dma_start(out=g8t_sb[:], in_=g8t_d[:, :])

            ones1_sb = consts.tile([1, 128], f32)
            nc.vector.memset(ones1_sb[:], 1.0)
            ones_p32 = consts.tile([33, 32], f32)
            nc.vector.memset(ones_p32[:], 1.0)
            eps_sb = consts.tile([16, 1], f32)
            nc.vector.memset(eps_sb[:], EPS)

            # ---------- GroupNorm: per-channel affine xn = A*x + B ----------
            xn_sb = data.tile([128, NT, S], bf16)
            xnsl_sb = data.tile([128, NT, SL], bf16)
            a_sb = data.tile([128, NT, 1], f32)
            b_sb = data.tile([128, NT, 1], f32)

            from contextlib import ExitStack as _ES
            _gnqkv = _ES()
            ps_gn = _gnqkv.enter_context(
                tc.tile_pool(name="ps_gnqkv", bufs=1, space="PSUM"))
            if True:
                for t in range(NT):
                    stats6 = work.tile([128, 8, 6], f32, tag="stats6", bufs=2)
                    x_sg = x_sb[:, t, :].rearrange("p (n f) -> p n f", f=512)
                    for sg in range(8):
                        nc.vector.bn_stats(out=stats6[:, sg, :], in_=x_sg[:, sg, :])
                    mv = work.tile([128, 2], f32, tag="mv", bufs=2)
                    nc.vector.bn_aggr(out=mv[:], in_=stats6[:])
                    # st2 = [mean, E[x^2]] per partition
                    st2 = work.tile([128, 2], f32, tag="st2", bufs=2)
                    nc.vector.tensor_copy(out=st2[:, 0:1], in_=mv[:, 0:1])
                    m2 = work.tile([128, 1], f32, tag="m2", bufs=2)
                    nc.vector.tensor_mul(m2[:], mv[:, 0:1], mv[:, 0:1])
                    nc.vector.tensor_add(st2[:, 1:2], mv[:, 1:2], m2[:])
                    # group reduce: (16,2) = g8^T @ st2
                    gstat_ps = ps_gn.tile([16, 2], f32, tag="gstat", bufs=1)
                    nc.tensor.matmul(gstat_ps[:], g8_sb[:], st2[:], start=True, stop=True)
                    gs = work.tile([16, 2], f32, tag="gs", bufs=2)
                    nc.vector.tensor_copy(out=gs[:], in_=gstat_ps[:])
                    # var = E2 - mean^2 ; rstd = 1/sqrt(var+eps)
                    gm2 = work.tile([16, 1], f32, tag="gm2", bufs=2)
                    nc.vector.tensor_mul(gm2[:], gs[:, 0:1], gs[:, 0:1])
                    gvar = work.tile([16, 1], f32, tag="gvar", bufs=2)
                    nc.vector.tensor_tensor(
                        out=gvar[:], in0=gs[:, 1:2], in1=gm2[:], op=Alu.subtract
                    )
                    mr = work.tile([16, 2], f32, tag="mr", bufs=2)
                    nc.vector.tensor_copy(out=mr[:, 0:1], in_=gs[:, 0:1])
                    gstd = work.tile([16, 1], f32, tag="gstd", bufs=2)
                    nc.scalar.activation(
                        out=gstd[:], in_=gvar[:], func=Act.Sqrt, bias=eps_sb[:], scale=1.0
                    )
                    nc.vector.reciprocal(out=mr[:, 1:2], in_=gstd[:])
                    # broadcast (mean, rstd) back to 128 channels
                    bcast_ps = ps_gn.tile([128, 2], f32, tag="gbcast", bufs=1)
                    nc.tensor.matmul(bcast_ps[:], g8t_sb[:], mr[:], start=True, stop=True)
                    # A = rstd*w ; B = b - mean*A
                    nc.vector.tensor_mul(a_sb[:, t, :], bcast_ps[:, 1:2], gnw_sb[:, t, :])
                    tmp = work.tile([128, 1], f32, tag="tmpB", bufs=2)
                    nc.vector.tensor_mul(tmp[:], bcast_ps[:, 0:1], a_sb[:, t, :])
                    nc.vector.tensor_tensor(
                        out=b_sb[:, t, :], in0=gnb_sb[:, t, :], in1=tmp[:], op=Alu.subtract
                    )
                for t in range(NT):
                    nc.gpsimd.tensor_scalar(
                        out=xn_sb[:, t, :], in0=x_sb[:, t, :],
                        scalar1=a_sb[:, t, :], scalar2=b_sb[:, t, :],
                        op0=Alu.mult, op1=Alu.add,
                    )
                    nc.gpsimd.tensor_scalar(
                        out=xnsl_sb[:, t, :], in0=xsl_sb[:, t, :],
                        scalar1=a_sb[:, t, :], scalar2=b_sb[:, t, :],
                        op0=Alu.mult, op1=Alu.add,
                    )

            # ---------- QKV ----------
            # K: (256, 4096) rows = h*32+d (o-tile g holds heads 4g..4g+3)
            # Q: (256, 1024) same row layout, token slice only
            # VT: (4096, 8, 33) bf16; col 32 of each head block = 1.0 (denominator)
            kmat = data.tile([128, NT, S], bf16)
            qmat = data.tile([128, NT, SL], bf16)
            vt_sb = data.tile([128, TCH, NH, 33], bf16)
            nc.vector.memset(vt_sb[:, :, :, 32:33], 1.0)

            if True:
                ps_qkv = ps_gn
                for g in range(NT):
                    osl = slice(g * 128, (g + 1) * 128)
                    for j in range(S // 512):
                        ps_k = ps_qkv.tile([128, 512], f32, tag="ps_k", bufs=3)
                        for ci in range(NT):
                            nc.tensor.matmul(
                                ps_k[:],
                                wk_sb[:, ci, osl],
                                xn_sb[:, ci, j * 512:(j + 1) * 512],
                                start=(ci == 0), stop=(ci == NT - 1),
                            )
                        if j % 2 == 0:
                            nc.vector.tensor_scalar(
                                out=kmat[:, g, j * 512:(j + 1) * 512], in0=ps_k[:],
                                scalar1=bk_sb[:, g, :], scalar2=None,
                                op0=Alu.add,
                            )
                        else:
                            nc.scalar.add(
                                out=kmat[:, g, j * 512:(j + 1) * 512], in_=ps_k[:],
                                add=bk_sb[:, g, :],
                            )
                    for j in range(SL // 512):
                        ps_q = ps_qkv.tile([128, 512], f32, tag="ps_k", bufs=3)
                        for ci in range(NT):
                            nc.tensor.matmul(
                                ps_q[:],
                                wq_sb[:, ci, osl],
                                xnsl_sb[:, ci, j * 512:(j + 1) * 512],
                                start=(ci == 0), stop=(ci == NT - 1),
                            )
                        if j % 2 == 0:
                            nc.vector.tensor_scalar(
                                out=qmat[:, g, j * 512:(j + 1) * 512], in0=ps_q[:],
                                scalar1=bq_sb[:, g, :], scalar2=None,
                                op0=Alu.add,
                            )
                        else:
                            nc.scalar.add(
                                out=qmat[:, g, j * 512:(j + 1) * 512], in_=ps_q[:],
                                add=bq_sb[:, g, :],
                            )
                # V^T chunks: (128 t, 256 d) += xn_chunk^T @ wv ; + ones row @ bv_row
                for tch in range(TCH):
                    ps_vt = ps_qkv.tile([128, 256], f32, tag="ps_vt", bufs=2)
                    for ci in range(NT):
                        nc.tensor.matmul(
                            ps_vt[:],
                            xn_sb[:, ci, tch * 128:(tch + 1) * 128],
                            wv_sb[:, ci, :],
                            start=(ci == 0), stop=False,
                        )
                    nc.tensor.matmul(
                        ps_vt[:], ones1_sb[:], bvr_sb[:], start=False, stop=True,
                    )
                    if tch % 2 == 0:
                        nc.vector.tensor_copy(
                            out=vt_sb[:, tch, :, 0:32],
                            in_=ps_vt[:].rearrange("p (h d) -> p h d", d=32),
                        )
                    else:
                        nc.scalar.copy(
                            out=vt_sb[:, tch, :, 0:32],
                            in_=ps_vt[:].rearrange("p (h d) -> p h d", d=32),
                        )

            _gnqkv.close()

            # ---------- attention ----------
            # head pairs share one score tile: pair p = heads (hA, hB) of kmat
            # tile g at partition offsets (offA, offB); scores row-packed into
            # PE row groups, AV col-packed into PSUM col groups 0 / 2.
            exp4 = _register_exp4()
            out_un = data.tile([128, NT, SL], f32)
            den_sb = data.tile([33, NH, SL], f32)

            with tc.tile_pool(name="ps_att", bufs=1, space="PSUM") as ps_att:
                for g in range(NT):
                    h0 = g * 4  # heads h0..h0+3 in kmat tile g
                    for half in range(2):
                        s0_ = half * 512
                        qs = qmat[:, g, s0_:s0_ + 512]
                        # av0 holds heads h0 (cols 0..32) & h0+2 (64..96);
                        # av1 holds heads h0+1 & h0+3.
                        av0 = ps_att.tile([97, 512], f32, tag="ps_av", bufs=2)
                        av1 = ps_att.tile([97, 512], f32, tag="ps_av", bufs=2)
                        # software pipeline: scores/exp emitted LA chunks ahead
                        # of AV so the PE never stalls on the exp semaphore
                        LA = 1
                        ex_q = {}
                        for tt in range(TCH + LA):
                            if tt < TCH:
                                tch = tt
                                ks = slice(tch * 128, (tch + 1) * 128)
                                # 4-way row-packed scores: full PE array active
                                ps_a = ps_att.tile(
                                    [128, 1024], f32, tag="ps_sc", bufs=3
                                )
                                ps_b = ps_att.tile(
                                    [128, 1024], f32, tag="ps_sc", bufs=3
                                )
                                for r, (pst, col) in enumerate(
                                    ((ps_a, 0), (ps_a, 512), (ps_b, 0), (ps_b, 512))
                                ):
                                    rb = r * 32
                                    nc.tensor.matmul(
                                        pst[:, col:col + 512],
                                        kmat[rb:rb + 32, g, ks],
                                        qs[rb:rb + 32, :],
                                        start=True, stop=True,
                                        tile_position=(rb, 0),
                                    )
                                ex_a = work.tile([128, 1024], bf16, tag="ex", bufs=6)
                                ex_b = work.tile([128, 1024], bf16, tag="ex", bufs=6)
                                # ex_a (heads 0,1) on ACT; ex_b (heads 2,3) on
                                # DVE, except every 6th chunk -> ACT to balance
                                nc.scalar.activation(
                                    out=ex_a[:], in_=ps_a[:], func=Act.Exp, scale=4.0
                                )
                                if tch % 16 == 15:
                                    nc.scalar.activation(
                                        out=ex_b[:], in_=ps_b[:], func=Act.Exp,
                                        scale=4.0,
                                    )
                                else:
                                    nc.vector._custom_dve(
                                        exp4, out=ex_b[:], in0=ps_b[:],
                                        s0=EXP4_C0, s1=EXP4_C1,
                                    )
                                ex_q[tch] = (ex_a, ex_b)
                            if tt >= LA:
                                tch = tt - LA
                                ex_a, ex_b = ex_q.pop(tch)
                                first, last = tch == 0, tch == TCH - 1
                                for av, col, ex, xcol in (
                                    (av0, 0, ex_a, 0),        # head h0
                                    (av1, 0, ex_a, 512),      # head h0+1
                                    (av0, 64, ex_b, 0),       # head h0+2
                                    (av1, 64, ex_b, 512),     # head h0+3
                                ):
                                    h = h0 + (0 if col == 0 else 2) + (
                                        0 if av is av0 else 1
                                    )
                                    nc.tensor.matmul(
                                        av[col:col + 33, :],
                                        vt_sb[:, tch, h, :],
                                        ex[:, xcol:xcol + 512],
                                        start=first, stop=last,
                                        tile_position=(0, col),
                                        skip_group_check=True,
                                    )
                        # evacuate accumulators (quadrant-aligned partition moves)
                        for av, col, h in (
                            (av0, 0, h0), (av1, 0, h0 + 1),
                            (av0, 64, h0 + 2), (av1, 64, h0 + 3),
                        ):
                            ob = (h - h0) * 32
                            # out_un evacuation on ACT (keeps DVE free for exp)
                            nc.scalar.copy(
                                out=out_un[ob:ob + 32, g, s0_:s0_ + 512],
                                in_=av[col:col + 32, :],
                            )
                            nc.vector.tensor_copy(
                                out=den_sb[32:33, h, s0_:s0_ + 512],
                                in_=av[col + 32:col + 33, :],
                            )

            # ---------- normalize + projection + residual ----------
            attn_sb = data.tile([128, NT, SL], bf16)
            osl_sb = data.tile([128, NT, SL], f32)

            with tc.tile_pool(name="ps_prj", bufs=1, space="PSUM") as ps_prj:
                for g in range(NT):
                    # broadcast raw denominators to all 128 rows of the group tile
                    ps_bc = ps_prj.tile([128, SL], f32, tag="ps_bc", bufs=2)
                    for r in range(4):
                        h = g * 4 + r
                        for j in range(NP):
                            nc.tensor.matmul(
                                ps_bc[r * 32:(r + 1) * 32, j * 512:(j + 1) * 512],
                                ones_p32[32:33, :],
                                den_sb[32:33, h, j * 512:(j + 1) * 512],
                                start=True, stop=True,
                                tile_position=(32, r * 32),
                            )
                    rec = work.tile([128, SL], f32, tag="rec", bufs=2)
                    scr = work.tile([128, SL], f32, tag="scr", bufs=2)
                    nc.vector.reciprocal_approx_accurate(
                        out=rec[:], in_=ps_bc[:], scratch=scr[:]
                    )
                    nc.vector.tensor_mul(attn_sb[:, g, :], out_un[:, g, :], rec[:])
                for oi in range(NT):
                    for j in range(NP):
                        ps_p = ps_prj.tile([128, 512], f32, tag="ps_p", bufs=4)
                        for ci in range(NT):
                            nc.tensor.matmul(
                                ps_p[:],
                                p_sb[:, ci, oi * 128:(oi + 1) * 128],
                                attn_sb[:, ci, j * 512:(j + 1) * 512],
                                start=(ci == 0), stop=(ci == NT - 1),
                            )
                        # out = (psum + pb) + x_residual
                        nc.vector.scalar_tensor_tensor(
                            out=osl_sb[:, oi, j * 512:(j + 1) * 512],
                            in0=ps_p[:],
                            scalar=pb_sb[:, oi, :],
                            in1=xsl_sb[:, oi, j * 512:(j + 1) * 512],
                            op0=Alu.add, op1=Alu.add,
                        )
                for oi in range(NT):
                    nc.sync.dma_start(
                        out=out_d[oi * 128:(oi + 1) * 128, :], in_=osl_sb[:, oi, :]
                    )

    nc.compile()
    return nc


def get_program():
    global _PROGRAM
    if _PROGRAM is None:
        _PROGRAM = _build_program()
    return _PROGRAM


def make_in_maps(x, gn_w, gn_b, qkv_w, qkv_b, proj_w, proj_b):
    """Host-side prep: slice/transpose/cast the small weights, shard x."""
    x = np.asarray(x, dtype=np.float32)
    xf = x.reshape(B, C, S)
    # extra 1/4: scores are stored pre-scaled by 1/4 (ACT exp uses scale=4,
    # the DVE poly evaluates exp(4s) directly)
    scale = 1.0 / (4.0 * math.sqrt(HD))

    qkv_w = np.asarray(qkv_w, dtype=np.float32)
    qkv_b = np.asarray(qkv_b, dtype=np.float32)
    wq = (qkv_w[0:C] * scale).T.astype(BF16)          # (c, o)
    wk = qkv_w[C:2 * C].T.astype(BF16)
    wv = qkv_w[2 * C:3 * C].T.astype(BF16)
    pt = np.asarray(proj_w, dtype=np.float32).T.astype(BF16)
    bq = (qkv_b[0:C] * scale).reshape(C, 1).astype(np.float32)
    bk = qkv_b[C:2 * C].reshape(C, 1).astype(np.float32)
    bvr = qkv_b[2 * C:3 * C].reshape(1, C).astype(np.float32)
    pb = np.asarray(proj_b, dtype=np.float32).reshape(C, 1)
    gnw = np.asarray(gn_w, dtype=np.float32).reshape(C, 1)
    gnb = np.asarray(gn_b, dtype=np.float32).reshape(C, 1)

    g8 = np.zeros((128, 16), np.float32)
    g8t = np.zeros((16, 128), np.float32)
    for p in range(128):
        g8[p, p // CPG] = 1.0 / CPG
        g8t[p // CPG, p] = 1.0
    common = dict(
        wq_t=wq, wk_t=wk, wv_t=wv, p_t=pt, bq=bq, bk=bk, bv_row=bvr, pb=pb,
        gnw=gnw, gnb=gnb, g8=g8, g8t=g8t,
    )
    in_maps = []
    for core in range(NCORES):
        bi, sl = core // NSL, core % NSL
        m = dict(common)
        m["x_full"] = np.ascontiguousarray(xf[bi])
        m["x_sl"] = np.ascontiguousarray(xf[bi][:, sl * SL:(sl + 1) * SL])
        in_maps.append(m)
    return in_maps


def kernel(x, gn_w, gn_b, qkv_w, qkv_b, proj_w, proj_b):
    global LAST_RESULTS
    from concourse.bass_utils import run_bass_kernel_spmd

    nc = get_program()
    in_maps = make_in_maps(x, gn_w, gn_b, qkv_w, qkv_b, proj_w, proj_b)
    res = run_bass_kernel_spmd(nc, in_maps, list(range(NCORES)))
    LAST_RESULTS = res
    out = np.empty((B, C, S), np.float32)
    for core in range(NCORES):
        bi, sl = core // NSL, core % NSL
        out[bi][:, sl * SL:(sl + 1) * SL] = res.results[core]["out_sl"]
    return out.reshape(B, C, H, W).astype(np.float32)


# revision 15
# speedup vs baseline: 1.1434x; 1.1434x over previous
"""Trainium2 Bass kernel for nn_AttentionBlock (GroupNorm -> MHA(8 heads, s=4096) -> proj -> residual).

Sharding: 8 cores = 2 batches x 4 query-token slices (1024 tokens each).
Each core computes GroupNorm + full K/V for its batch (redundant across the
4 cores of a batch), Q only for its token slice, streaming softmax attention
for all 8 heads over its slice, projection + residual for its slice.
Output slices are disjoint -> no cross-core reduction.

Self-contained: hardcodes shapes (x: (2,256,64,64) f32) and reads nothing
from /root/problem.
"""

import os
import sys
import math
import numpy as np

sys.path.insert(0, "/opt/trn_rl_repo")

import ml_dtypes  # noqa: E402

BF16 = ml_dtypes.bfloat16

# ---- problem constants (hardcoded) ----
B, C, H, W = 2, 256, 64, 64
S = H * W            # 4096 tokens
NH, HD = 8, 32       # heads, head dim
GROUPS = 32          # groupnorm groups
CPG = C // GROUPS    # 8 channels / group
EPS = 1e-5
NCORES = 8
NSL = 4              # token slices per batch
SL = S // NSL        # 1024 tokens per core
NT = C // 128        # 2 channel tiles
TCH = S // 128       # 32 key/t chunks
NP = SL // 512       # 2 moving pieces per psum row-block

_PROGRAM = None      # (nc, meta) cache
LAST_RESULTS = None  # BassKernelResults of last run (for test introspection)

# softmax exp split between ACT (hw spline) and a custom DVE cubic-poly op.
# scores are stored pre-scaled by 1/4; ACT applies scale=4, the DVE poly
# evaluates exp(4s) ~= (1 + s(1 + s(c1 + s*c0)))^4 (fit on |score|<=0.85).
EXP4_C0 = 0.16665777
EXP4_C1 = 0.5015184
DVE_EXP_PAT = (0, 2, 4, 6)   # chunk c goes to DVE iff c % 9 in this set
_EXP4_OP = None


def _register_exp4():
    """Register the ANT_EXP4 custom DVE op (runtime append to dve_ops.OPS)."""
    global _EXP4_OP
    if _EXP4_OP is not None:
        return _EXP4_OP
    from concourse import dve_ops
    from concourse.dve_spec import Spec, Src0, C0, C1, One, lower
    from concourse.dve_uop import DveOpSpec
    from concourse.dve_ops import DveOp

    for op in dve_ops.OPS:
        if op.name == "ANT_EXP4":
            _EXP4_OP = op
            return op

    def _ref(in0, in1, s0, s1, imm2):
        t1 = s1 + in0 * s0
        t2 = 1.0 + in0 * t1
        t3 = 1.0 + in0 * t2
        t4 = t3 * t3
        return (t4 * t4).astype(np.float32)

    t1 = C1 + Src0 * C0
    t2 = One + Src0 * t1
    t3 = One + Src0 * t2
    t4 = t3 * t3
    spec = Spec(body=t4 * t4, reference=_ref)
    shas = {
        ver: DveOpSpec(
            name="ANT_EXP4", opcode=0, uops=lower(spec, ver=ver), rd1_en=False
        ).sha(ver)
        for ver in ("v3", "v4")
    }
    op = DveOp("ANT_EXP4", spec, subdim=False, uops_sha=shas)
    dve_ops.OPS.append(op)
    dve_ops._SUB_OPCODE_FOR_NAME[op.name] = (
        dve_ops._CUSTOM_DVE_ROW_BASE + len(dve_ops.OPS) - 1
    )
    dve_ops.CUSTOM_DVE_SPECS[op.name] = spec
    _EXP4_OP = op
    return op


def _build_program():
    import concourse.bass as bass
    import concourse.tile as tile
    from concourse import bacc, mybir

    f32 = mybir.dt.float32
    bf16 = mybir.dt.bfloat16
    Alu = mybir.AluOpType
    Act = mybir.ActivationFunctionType

    nc = bacc.Bacc(
        "TRN2",
        target_bir_lowering=False,
        debug=False,
        enable_asserts=False,
        num_devices=NCORES,
    )

    # ---- DRAM I/O ----
    x_full = nc.dram_tensor("x_full", [C, S], f32, kind="ExternalInput").ap()
    x_sl = nc.dram_tensor("x_sl", [C, SL], f32, kind="ExternalInput").ap()
    wq_t = nc.dram_tensor("wq_t", [C, C], bf16, kind="ExternalInput").ap()
    wk_t = nc.dram_tensor("wk_t", [C, C], bf16, kind="ExternalInput").ap()
    wv_t = nc.dram_tensor("wv_t", [C, C], bf16, kind="ExternalInput").ap()
    p_t = nc.dram_tensor("p_t", [C, C], bf16, kind="ExternalInput").ap()
    bq_d = nc.dram_tensor("bq", [C, 1], f32, kind="ExternalInput").ap()
    bk_d = nc.dram_tensor("bk", [C, 1], f32, kind="ExternalInput").ap()
    pb_d = nc.dram_tensor("pb", [C, 1], f32, kind="ExternalInput").ap()
    gnw_d = nc.dram_tensor("gnw", [C, 1], f32, kind="ExternalInput").ap()
    gnb_d = nc.dram_tensor("gnb", [C, 1], f32, kind="ExternalInput").ap()
    g8_d = nc.dram_tensor("g8", [128, 16], f32, kind="ExternalInput").ap()
    g8t_d = nc.dram_tensor("g8t", [16, 128], f32, kind="ExternalInput").ap()
    out_d = nc.dram_tensor("out_sl", [C, SL], f32, kind="ExternalOutput").ap()

    with tile.TileContext(nc) as tc:
        with tc.tile_pool(name="consts", bufs=1) as consts, \
             tc.tile_pool(name="data", bufs=1) as data, \
             tc.tile_pool(name="work", bufs=3) as work:

            # ---------- load inputs ----------
            x_sb = data.tile([128, NT, S], f32)
            xsl_sb = data.tile([128, NT, SL], f32)
            for t in range(NT):
                nc.sync.dma_start(out=x_sb[:, t, :], in_=x_full[t * 128:(t + 1) * 128, :])
                nc.sync.dma_start(out=xsl_sb[:, t, :], in_=x_sl[t * 128:(t + 1) * 128, :])

            wq_sb = consts.tile([128, NT, C], bf16)
            wk_sb = consts.tile([128, NT, C], bf16)
            wv_sb = consts.tile([128, NT, C], bf16)
            p_sb = consts.tile([128, NT, C], bf16)
            bq_sb = consts.tile([128, NT, 1], f32)
            bk_sb = consts.tile([128, NT, 1], f32)
            pb_sb = consts.tile([128, NT, 1], f32)
            gnw_sb = consts.tile([128, NT, 1], f32)
            gnb_sb = consts.tile([128, NT, 1], f32)
            # one DMA per tensor (t-major source view), on the SWDGE queue so
            # they flow in parallel with the big x loads on the HWDGE queue
            for dst, srcd in ((wq_sb, wq_t), (wk_sb, wk_t), (wv_sb, wv_t),
                              (p_sb, p_t)):
                nc.gpsimd.dma_start(
                    out=dst[:], in_=srcd.rearrange("(t p) c -> p t c", p=128)
                )
            for dst, srcd in ((bq_sb, bq_d), (bk_sb, bk_d), (pb_sb, pb_d),
                              (gnw_sb, gnw_d), (gnb_sb, gnb_d)):
                nc.gpsimd.dma_start(
                    out=dst[:], in_=srcd.rearrange("(t p) c -> p t c", p=128)
                )
            g8_sb = consts.tile([128, 16], f32)
            nc.gpsimd.dma_start(out=g8_sb[:], in_=g8_d[:, :])
            g8t_sb = consts.tile([16, 128], f32)
            nc.gpsimd.dma_start(out=g8t_sb[:], in_=g8t_d[:, :])

            ones_p32 = consts.tile([33, 32], f32)
            nc.vector.memset(ones_p32[:], 1.0)
            eps_sb = consts.tile([16, 1], f32)
            nc.vector.memset(eps_sb[:], EPS)

            # ---------- GroupNorm: per-channel affine xn = A*x + B ----------
            xn_sb = data.tile([128, NT, S], bf16)
            xnsl_sb = data.tile([128, NT, SL], bf16)
            a_sb = data.tile([128, NT, 1], f32)
            b_sb = data.tile([128, NT, 1], f32)

            from contextlib import ExitStack as _ES
            _gnqkv = _ES()
            ps_gn = _gnqkv.enter_context(
                tc.tile_pool(name="ps_gnqkv", bufs=1, space="PSUM"))
            if True:
                for t in range(NT):
                    stats6 = work.tile([128, 8, 6], f32, tag="stats6", bufs=2)
                    x_sg = x_sb[:, t, :].rearrange("p (n f) -> p n f", f=512)
                    for sg in range(8):
                        nc.vector.bn_stats(out=stats6[:, sg, :], in_=x_sg[:, sg, :])
                    mv = work.tile([128, 2], f32, tag="mv", bufs=2)
                    nc.vector.bn_aggr(out=mv[:], in_=stats6[:])
                    # st2 = [mean, E[x^2]] per partition
                    st2 = work.tile([128, 2], f32, tag="st2", bufs=2)
                    nc.vector.tensor_copy(out=st2[:, 0:1], in_=mv[:, 0:1])
                    m2 = work.tile([128, 1], f32, tag="m2", bufs=2)
                    nc.vector.tensor_mul(m2[:], mv[:, 0:1], mv[:, 0:1])
                    nc.vector.tensor_add(st2[:, 1:2], mv[:, 1:2], m2[:])
                    # group reduce: (16,2) = g8^T @ st2
                    gstat_ps = ps_gn.tile([16, 2], f32, tag="gstat", bufs=1)
                    nc.tensor.matmul(gstat_ps[:], g8_sb[:], st2[:], start=True, stop=True)
                    gs = work.tile([16, 2], f32, tag="gs", bufs=2)
                    nc.vector.tensor_copy(out=gs[:], in_=gstat_ps[:])
                    # var = E2 - mean^2 ; rstd = 1/sqrt(var+eps)
                    gm2 = work.tile([16, 1], f32, tag="gm2", bufs=2)
                    nc.vector.tensor_mul(gm2[:], gs[:, 0:1], gs[:, 0:1])
                    gvar = work.tile([16, 1], f32, tag="gvar", bufs=2)
                    nc.vector.tensor_tensor(
                        out=gvar[:], in0=gs[:, 1:2], in1=gm2[:], op=Alu.subtract
                    )
                    mr = work.tile([16, 2], f32, tag="mr", bufs=2)
                    nc.vector.tensor_copy(out=mr[:, 0:1], in_=gs[:, 0:1])
                    gstd = work.tile([16, 1], f32, tag="gstd", bufs=2)
                    nc.scalar.activation(
                        out=gstd[:], in_=gvar[:], func=Act.Sqrt, bias=eps_sb[:], scale=1.0
                    )
                    nc.vector.reciprocal(out=mr[:, 1:2], in_=gstd[:])
                    # broadcast (mean, rstd) back to 128 channels
                    bcast_ps = ps_gn.tile([128, 2], f32, tag="gbcast", bufs=1)
                    nc.tensor.matmul(bcast_ps[:], g8t_sb[:], mr[:], start=True, stop=True)
                    # A = rstd*w ; B = b - mean*A
                    nc.vector.tensor_mul(a_sb[:, t, :], bcast_ps[:, 1:2], gnw_sb[:, t, :])
                    tmp = work.tile([128, 1], f32, tag="tmpB", bufs=2)
                    nc.vector.tensor_mul(tmp[:], bcast_ps[:, 0:1], a_sb[:, t, :])
                    nc.vector.tensor_tensor(
                        out=b_sb[:, t, :], in0=gnb_sb[:, t, :], in1=tmp[:], op=Alu.subtract
                    )
                for t in range(NT):
                    nc.vector.tensor_scalar(
                        out=xn_sb[:, t, :], in0=x_sb[:, t, :],
                        scalar1=a_sb[:, t, :], scalar2=b_sb[:, t, :],
                        op0=Alu.mult, op1=Alu.add,
                    )
                    nc.vector.tensor_scalar(
                        out=xnsl_sb[:, t, :], in0=xsl_sb[:, t, :],
                        scalar1=a_sb[:, t, :], scalar2=b_sb[:, t, :],
                        op0=Alu.mult, op1=Alu.add,
                    )

            # ---------- QKV ----------
            # K: (256, 4096) rows = h*32+d (o-tile g holds heads 4g..4g+3)
            # Q: (256, 1024) same row layout, token slice only
            # VT: (4096, 8, 33) bf16; col 32 of each head block = 1.0 (denominator)
            kmat = data.tile([128, NT, S], bf16)
            qmat = data.tile([128, NT, SL], bf16)
            vt_sb = data.tile([128, TCH, NH, 33], bf16)
            nc.vector.memset(vt_sb[:, :, :, 32:33], 1.0)

            if True:
                ps_qkv = ps_gn
                for g in range(NT):
                    osl = slice(g * 128, (g + 1) * 128)
                    for j in range(S // 512):
                        ps_k = ps_qkv.tile([128, 512], f32, tag="ps_k", bufs=3)
                        for ci in range(NT):
                            nc.tensor.matmul(
                                ps_k[:],
                                wk_sb[:, ci, osl],
                                xn_sb[:, ci, j * 512:(j + 1) * 512],
                                start=(ci == 0), stop=(ci == NT - 1),
                            )
                        if j % 2 == 0:
                            nc.vector.tensor_scalar(
                                out=kmat[:, g, j * 512:(j + 1) * 512], in0=ps_k[:],
                                scalar1=bk_sb[:, g, :], scalar2=None,
                                op0=Alu.add,
                            )
                        else:
                            nc.scalar.add(
                                out=kmat[:, g, j * 512:(j + 1) * 512], in_=ps_k[:],
                                add=bk_sb[:, g, :],
                            )
                    for j in range(SL // 512):
                        ps_q = ps_qkv.tile([128, 512], f32, tag="ps_k", bufs=3)
                        for ci in range(NT):
                            nc.tensor.matmul(
                                ps_q[:],
                                wq_sb[:, ci, osl],
                                xnsl_sb[:, ci, j * 512:(j + 1) * 512],
                                start=(ci == 0), stop=(ci == NT - 1),
                            )
                        if j % 2 == 0:
                            nc.vector.tensor_scalar(
                                out=qmat[:, g, j * 512:(j + 1) * 512], in0=ps_q[:],
                                scalar1=bq_sb[:, g, :], scalar2=None,
                                op0=Alu.add,
                            )
                        else:
                            nc.scalar.add(
                                out=qmat[:, g, j * 512:(j + 1) * 512], in_=ps_q[:],
                                add=bq_sb[:, g, :],
                            )
                # V^T chunks: (128 t, 256 d) += xn_chunk^T @ wv ; + ones row @ bv_row
                for tch in range(TCH):
                    ps_vt = ps_qkv.tile([128, 256], f32, tag="ps_vt", bufs=2)
                    for ci in range(NT):
                        nc.tensor.matmul(
                            ps_vt[:],
                            xn_sb[:, ci, tch * 128:(tch + 1) * 128],
                            wv_sb[:, ci, :],
                            start=(ci == 0), stop=(ci == NT - 1),
                        )
                    if tch % 2 == 0:
                        nc.vector.tensor_copy(
                            out=vt_sb[:, tch, :, 0:32],
                            in_=ps_vt[:].rearrange("p (h d) -> p h d", d=32),
                        )
                    else:
                        nc.scalar.copy(
                            out=vt_sb[:, tch, :, 0:32],
                            in_=ps_vt[:].rearrange("p (h d) -> p h d", d=32),
                        )

            _gnqkv.close()

            # ---------- attention ----------
            # head pairs share one score tile: pair p = heads (hA, hB) of kmat
            # tile g at partition offsets (offA, offB); scores row-packed into
            # PE row groups, AV col-packed into PSUM col groups 0 / 2.
            exp4 = _register_exp4()
            out_un = data.tile([128, NT, SL], f32)
            den_sb = data.tile([33, NH, SL], f32)

            with tc.tile_pool(name="ps_att", bufs=1, space="PSUM") as ps_att:
                for g in range(NT):
                    h0 = g * 4  # heads h0..h0+3 in kmat tile g
                    for half in range(2):
                        s0_ = half * 512
                        qs = qmat[:, g, s0_:s0_ + 512]
                        # av0 holds heads h0 (cols 0..32) & h0+2 (64..96);
                        # av1 holds heads h0+1 & h0+3.
                        av0 = ps_att.tile([97, 512], f32, tag="ps_av", bufs=2)
                        av1 = ps_att.tile([97, 512], f32, tag="ps_av", bufs=2)
                        # software pipeline: scores/exp emitted LA chunks ahead
                        # of AV so the PE never stalls on the exp semaphore
                        LA = 1
                        ex_q = {}
                        for tt in range(TCH + LA):
                            if tt < TCH:
                                tch = tt
                                ks = slice(tch * 128, (tch + 1) * 128)
                                # 4-way row-packed scores: full PE array active
                                ps_a = ps_att.tile(
                                    [128, 1024], f32, tag="ps_sc", bufs=3
                                )
                                ps_b = ps_att.tile(
                                    [128, 1024], f32, tag="ps_sc", bufs=3
                                )
                                for r, (pst, col) in enumerate(
                                    ((ps_a, 0), (ps_a, 512), (ps_b, 0), (ps_b, 512))
                                ):
                                    rb = r * 32
                                    nc.tensor.matmul(
                                        pst[:, col:col + 512],
                                        kmat[rb:rb + 32, g, ks],
                                        qs[rb:rb + 32, :],
                                        start=True, stop=True,
                                        tile_position=(rb, 0),
                                    )
                                ex_a = work.tile([128, 1024], bf16, tag="ex", bufs=6)
                                ex_b = work.tile([128, 1024], bf16, tag="ex", bufs=6)
                                # ex_a (heads 0,1) on ACT; ex_b (heads 2,3) on
                                # DVE, except every 6th chunk -> ACT to balance
                                nc.scalar.activation(
                                    out=ex_a[:], in_=ps_a[:], func=Act.Exp, scale=4.0
                                )
                                if tch % 16 == 15:
                                    nc.scalar.activation(
                                        out=ex_b[:], in_=ps_b[:], func=Act.Exp,
                                        scale=4.0,
                                    )
                                else:
                                    nc.vector._custom_dve(
                                        exp4, out=ex_b[:], in0=ps_b[:],
                                        s0=EXP4_C0, s1=EXP4_C1,
                                    )
                                ex_q[tch] = (ex_a, ex_b)
                            if tt >= LA:
                                tch = tt - LA
                                ex_a, ex_b = ex_q.pop(tch)
                                first, last = tch == 0, tch == TCH - 1
                                for av, col, ex, xcol in (
                                    (av0, 0, ex_a, 0),        # head h0
                                    (av1, 0, ex_a, 512),      # head h0+1
                                    (av0, 64, ex_b, 0),       # head h0+2
                                    (av1, 64, ex_b, 512),     # head h0+3
                                ):
                                    h = h0 + (0 if col == 0 else 2) + (
                                        0 if av is av0 else 1
                                    )
                                    nc.tensor.matmul(
                                        av[col:col + 33, :],
                                        vt_sb[:, tch, h, :],
                                        ex[:, xcol:xcol + 512],
                                        start=first, stop=last,
                                        tile_position=(0, col),
                                        skip_group_check=True,
                                    )
                        # evacuate accumulators (quadrant-aligned partition moves)
                        for av, col, h in (
                            (av0, 0, h0), (av1, 0, h0 + 1),
                            (av0, 64, h0 + 2), (av1, 64, h0 + 3),
                        ):
                            ob = (h - h0) * 32
                            # out_un evacuation on ACT (keeps DVE free for exp)
                            nc.scalar.copy(
                                out=out_un[ob:ob + 32, g, s0_:s0_ + 512],
                                in_=av[col:col + 32, :],
                            )
                            nc.vector.tensor_copy(
                                out=den_sb[32:33, h, s0_:s0_ + 512],
                                in_=av[col + 32:col + 33, :],
                            )

            # ---------- normalize + projection + residual ----------
            attn_sb = data.tile([128, NT, SL], bf16)
            osl_sb = data.tile([128, NT, SL], f32)

            with tc.tile_pool(name="ps_prj", bufs=1, space="PSUM") as ps_prj:
                for g in range(NT):
                    # broadcast raw denominators to all 128 rows of the group tile
                    ps_bc = ps_prj.tile([128, SL], f32, tag="ps_bc", bufs=2)
                    for r in range(4):
                        h = g * 4 + r
                        for j in range(NP):
                            nc.tensor.matmul(
                                ps_bc[r * 32:(r + 1) * 32, j * 512:(j + 1) * 512],
                                ones_p32[32:33, :],
                                den_sb[32:33, h, j * 512:(j + 1) * 512],
                                start=True, stop=True,
                                tile_position=(32, r * 32),
                            )
                    rec = work.tile([128, SL], f32, tag="rec", bufs=2)
                    scr = work.tile([128, SL], f32, tag="scr", bufs=2)
                    nc.vector.reciprocal_approx_accurate(
                        out=rec[:], in_=ps_bc[:], scratch=scr[:]
                    )
                    nc.vector.tensor_mul(attn_sb[:, g, :], out_un[:, g, :], rec[:])
                for oi in range(NT):
                    for j in range(NP):
                        ps_p = ps_prj.tile([128, 512], f32, tag="ps_p", bufs=4)
                        for ci in range(NT):
                            nc.tensor.matmul(
                                ps_p[:],
                                p_sb[:, ci, oi * 128:(oi + 1) * 128],
                                attn_sb[:, ci, j * 512:(j + 1) * 512],
                                start=(ci == 0), stop=(ci == NT - 1),
                            )
                        # out = (psum + pb) + x_residual
                        nc.vector.scalar_tensor_tensor(
                            out=osl_sb[:, oi, j * 512:(j + 1) * 512],
                            in0=ps_p[:],
                            scalar=pb_sb[:, oi, :],
                            in1=xsl_sb[:, oi, j * 512:(j + 1) * 512],
                            op0=Alu.add, op1=Alu.add,
                        )
                for oi in range(NT):
                    nc.sync.dma_start(
                        out=out_d[oi * 128:(oi + 1) * 128, :], in_=osl_sb[:, oi, :]
                    )

    nc.compile()
    return nc


def get_program():
    global _PROGRAM
    if _PROGRAM is None:
        _PROGRAM = _build_program()
    return _PROGRAM


def make_in_maps(x, gn_w, gn_b, qkv_w, qkv_b, proj_w, proj_b):
    """Host-side prep: slice/transpose/cast the small weights, shard x."""
    x = np.asarray(x, dtype=np.float32)
    xf = x.reshape(B, C, S)
    # extra 1/4: scores are stored pre-scaled by 1/4 (ACT exp uses scale=4,
    # the DVE poly evaluates exp(4s) directly)
    scale = 1.0 / (4.0 * math.sqrt(HD))

    qkv_w = np.asarray(qkv_w, dtype=np.float32)
    qkv_b = np.asarray(qkv_b, dtype=np.float32)
    wq = (qkv_w[0:C] * scale).T.astype(BF16)          # (c, o)
    wk = qkv_w[C:2 * C].T.astype(BF16)
    wv = qkv_w[2 * C:3 * C].T.astype(BF16)
    pt = np.asarray(proj_w, dtype=np.float32).T.astype(BF16)
    bq = (qkv_b[0:C] * scale).reshape(C, 1).astype(np.float32)
    bk = qkv_b[C:2 * C].reshape(C, 1).astype(np.float32)
    # V bias: softmax weights sum to 1, so +bv on V adds bv to each head's
    # output; fold proj_w @ bv into the projection bias instead.
    pw = np.asarray(proj_w, dtype=np.float32)
    pb = (np.asarray(proj_b, dtype=np.float32)
          + pw @ qkv_b[2 * C:3 * C]).reshape(C, 1)
    gnw = np.asarray(gn_w, dtype=np.float32).reshape(C, 1)
    gnb = np.asarray(gn_b, dtype=np.float32).reshape(C, 1)

    g8 = np.zeros((128, 16), np.float32)
    g8t = np.zeros((16, 128), np.float32)
    for p in range(128):
        g8[p, p // CPG] = 1.0 / CPG
        g8t[p // CPG, p] = 1.0
    common = dict(
        wq_t=wq, wk_t=wk, wv_t=wv, p_t=pt, bq=bq, bk=bk, pb=pb,
        gnw=gnw, gnb=gnb, g8=g8, g8t=g8t,
    )
    in_maps = []
    for core in range(NCORES):
        bi, sl = core // NSL, core % NSL
        m = dict(common)
        m["x_full"] = np.ascontiguousarray(xf[bi])
        m["x_sl"] = np.ascontiguousarray(xf[bi][:, sl * SL:(sl + 1) * SL])
        in_maps.append(m)
    return in_maps


def kernel(x, gn_w, gn_b, qkv_w, qkv_b, proj_w, proj_b):
    global LAST_RESULTS
    from concourse.bass_utils import run_bass_kernel_spmd

    nc = get_program()
    in_maps = make_in_maps(x, gn_w, gn_b, qkv_w, qkv_b, proj_w, proj_b)
    res = run_bass_kernel_spmd(nc, in_maps, list(range(NCORES)))
    LAST_RESULTS = res
    out = np.empty((B, C, S), np.float32)
    for core in range(NCORES):
        bi, sl = core // NSL, core % NSL
        out[bi][:, sl * SL:(sl + 1) * SL] = res.results[core]["out_sl"]
    return out.reshape(B, C, H, W).astype(np.float32)
